# revision 1
# baseline (speedup 1.0000x reference)
"""Trainium2 Bass kernel for a 2-layer LSTM decoder (B=128, T=32, F=2048,
E=512, H=1024, V=10000), data-parallel over batch across 8 NeuronCores.

Per-core plan (batch shard BL=16):
 - All matmuls use mode "activations stationary / weights streaming":
   out[batch, feat] = lhsT(=acts.T [K, BL]).T @ rhs(=W.T [K, N]).
   Stream cost is independent of batch size, so DP costs nothing on the
   sequential recurrence while sharding everything else 8x.
 - fp16 matmul operands (1 cycle/row on PE vs 4 for fp32), fp32 PSUM and
   fp32 cell state / gate activations.
 - whh0.T and whh1.T are SBUF-resident across all 32 steps; wih1.T is
   streamed from HBM each step in quarter-G pieces; wih0 is pre-applied to
   all timesteps at once (X0 = emb @ wih0.T + b0) through a DRAM scratch.
 - Biases are folded in with K=1 ones-row matmuls; gate order is
   host-permuted to [i, f, o, g]; init_h/init_c weights host-permuted to
   layer-major so h0/c0 split into layers by column range.
"""

import numpy as np

import concourse.bass as bass
import concourse.mybir as mybir
from concourse import bacc
from concourse.bass import MemorySpace
from concourse.bass_utils import run_bass_kernel_spmd
from concourse.masks import make_identity
from concourse.tile import TileContext

P = 128
NCORES = 8
B, T, F, E, H, L, V = 128, 32, 2048, 512, 1024, 2, 10000
G = 4 * H
BL = B // NCORES          # 16 batch rows per core
TB = T * BL               # 512 (t, b) rows per core
HL = H * L
KF, KE, KH = F // P, E // P, H // P      # 16, 4, 8
NB = G // 512             # 8 psum 512-col chunks across G
F16 = mybir.dt.float16
F32 = mybir.dt.float32

_cache = {}


def _build_nc(phases="ABCDE", fake_gather=False, d_mode=0):
    nc = bacc.Bacc("TRN2", target_bir_lowering=False, debug=False,
                   enable_asserts=False, num_devices=NCORES)

    dram = {}

    def din(name, shape, dt=F16):
        dram[name] = nc.dram_tensor(name, shape, dt, kind="ExternalInput").ap()
        return dram[name]

    featT = din("featT", [F, BL])
    emb_idx = din("emb_idx", [TB, 1], mybir.dt.int32)
    table = din("table", [V, E])
    init_hw = din("init_hw", [F, HL])
    init_cw = din("init_cw", [F, HL])
    init_hb = din("init_hb", [1, HL])
    init_cb = din("init_cb", [1, HL])
    wih0T = din("wih0T", [E, G])
    whh0T = din("whh0T", [H, G])
    wih1T = din("wih1T", [G, 1024])  # quarter-major: row q*H + h
    whh1T = din("whh1T", [H, G])
    bsum0 = din("bsum0", [1, G])
    bsum1 = din("bsum1", [1, G])
    fcwT = din("fcwT", [H, V])
    fcb_rep = din("fcb_rep", [P, V], F32)

    out = nc.dram_tensor("out", [TB, V], F32, kind="ExternalOutput").ap()
    x0buf = nc.dram_tensor("x0buf", [TB, G], F16, kind="Internal").ap()

    # DRAM views with the partition dim split out: row r = k*P + p
    featT_v = featT.rearrange("(k p) b -> p k b", p=P)
    init_hw_v = init_hw.rearrange("(k p) n -> p k n", p=P)
    init_cw_v = init_cw.rearrange("(k p) n -> p k n", p=P)
    wih0T_v = wih0T.rearrange("(k p) g -> p k g", p=P)
    whh0T_v = whh0T.rearrange("(k p) g -> p k g", p=P)
    wih1T_v = wih1T.rearrange("(q k p) n -> p q k n", q=4, p=P)
    whh1T_v = whh1T.rearrange("(k p) g -> p k g", p=P)
    fcwT_v = fcwT.rearrange("(k p) v -> p k v", p=P)
    idx_v = emb_idx.rearrange("(g p) one -> p g one", p=P)

    SIG = mybir.ActivationFunctionType.Sigmoid
    TANH = mybir.ActivationFunctionType.Tanh

    with TileContext(nc) as tc:
        with tc.tile_pool(name="const", bufs=1) as constp, \
             tc.tile_pool(name="resident", bufs=1) as resp, \
             tc.tile_pool(name="state", bufs=1) as statep, \
             tc.tile_pool(name="h0t", bufs=2) as h0tp, \
             tc.tile_pool(name="ys", bufs=1) as ysp:

            id128 = constp.tile([P, P], F16)
            make_identity(nc, id128)
            id16 = constp.tile([BL, BL], F16)
            make_identity(nc, id16)
            ones16 = constp.tile([1, BL], F16)
            nc.gpsimd.memset(ones16, 1.0)
            ones128 = constp.tile([1, P], F16)
            nc.gpsimd.memset(ones128, 1.0)
            bsum1_s = constp.tile([1, G], F16)
            nc.sync.dma_start(bsum1_s, bsum1)

            # SBUF-resident recurrent weights (64 KB/partition each)
            whh0_s = resp.tile([P, KH, G], F16)
            nc.sync.dma_start(whh0_s, whh0T_v)
            whh1_s = resp.tile([P, KH, G], F16)
            nc.sync.dma_start(whh1_s, whh1T_v)

            # Long-lived state
            c0_s = statep.tile([BL, H], F32)
            c1_s = statep.tile([BL, H], F32)
            h1T0_s = statep.tile([P, KH, BL], F16)   # t=0 layer-1 h.T
            # all layer-1 h.T outputs (feeds both recurrence and FC)
            ysT = ysp.tile([P, KH, T, BL], F16)

            # ---------------- Phases A-C scratch ---------------------------
            abc_pool = tc.alloc_tile_pool(name="embT", bufs=1)
            embT_s = abc_pool.tile([P, KE, TB], F16)  # transposed embeddings

            # ---------------- Phase A: embedding gather + transpose -------
            with tc.tile_pool(name="embp", bufs=2) as embp, \
                 tc.tile_pool(name="embpsum", bufs=2, space="PSUM") as embps:
                for g in range(TB // P):
                    if "A" not in phases:
                        break
                    rows = embp.tile([P, E], F16, tag="rows")
                    if fake_gather:
                        nc.sync.dma_start(rows, table[g * P : (g + 1) * P, :])
                    else:
                        idx_t = embp.tile([P, 1, 1], mybir.dt.int32, tag="idx")
                        nc.sync.dma_start(idx_t, idx_v[:, g : g + 1, :])
                        nc.gpsimd.indirect_dma_start(
                            out=rows[:],
                            out_offset=None,
                            in_=table[:],
                            in_offset=bass.IndirectOffsetOnAxis(
                                ap=idx_t[:, 0, :], axis=0
                            ),
                        )
                    pt = embps.tile([P, KE, P], F16, tag="pt")
                    for ke in range(KE):
                        nc.tensor.transpose(
                            pt[:, ke, :], rows[:, ke * P : (ke + 1) * P], id128
                        )
                    nc.vector.tensor_copy(
                        embT_s[:, :, g * P : (g + 1) * P], pt
                    )

            # ---------------- Phase B: h0/c0 init --------------------------
            with tc.tile_pool(name="initw", bufs=3) as initwp, \
                 tc.tile_pool(name="initsb", bufs=2) as initsb, \
                 tc.tile_pool(name="initpsum", bufs=1, space="PSUM") as initps:
                ihb_s = initsb.tile([1, HL], F16, tag="ib0")
                nc.sync.dma_start(ihb_s, init_hb)
                icb_s = initsb.tile([1, HL], F16, tag="ib1")
                nc.sync.dma_start(icb_s, init_cb)
                featT_s = initsb.tile([P, KF, BL], F16, tag="ft")
                nc.sync.dma_start(featT_s, featT_v)
                for which, (wv, bias_s) in enumerate(
                    ((init_hw_v, ihb_s), (init_cw_v, icb_s))
                ):
                    if "B" not in phases:
                        break
                    ps = initps.tile([BL, 4, 512], F32, tag="initps")
                    for k in range(KF):
                        wc = initwp.tile([P, 1, HL], F16, tag="iwc")
                        nc.sync.dma_start(wc, wv[:, k : k + 1, :])
                        for n in range(4):
                            nc.tensor.matmul(
                                ps[:, n, :],
                                featT_s[:, k, :],
                                wc[:, 0, n * 512 : (n + 1) * 512],
                                start=(k == 0),
                                stop=False,
                            )
                    for n in range(4):
                        nc.tensor.matmul(
                            ps[:, n, :],
                            ones16,
                            bias_s[:, n * 512 : (n + 1) * 512],
                            start=False,
                            stop=True,
                        )
                    if which == 0:
                        # h0: layer-major columns; cast to fp16, transpose
                        hh = initsb.tile([BL, HL], F16, tag="hh")
                        nc.vector.tensor_copy(hh, ps)
                        with tc.tile_pool(name="trps", bufs=2,
                                          space="PSUM") as trps:
                            for lay in range(L):
                                pt = trps.tile([P, KH, BL], F16, tag="pt")
                                for j in range(KH):
                                    nc.tensor.transpose(
                                        pt[:, j, :],
                                        hh[:, lay * H + j * P : lay * H + (j + 1) * P],
                                        id16,
                                    )
                                if lay == 0:
                                    h0T = h0tp.tile([P, KH, BL], F16, tag="h0T")
                                    nc.vector.tensor_copy(h0T, pt)
                                else:
                                    nc.vector.tensor_copy(h1T0_s, pt)
                    else:
                        nc.vector.tensor_copy(c0_s, ps[:, 0:2, :])
                        nc.vector.tensor_copy(c1_s, ps[:, 2:4, :])

            # ---------------- Phase C: X0 = emb @ wih0.T + b0 --------------
            with tc.tile_pool(name="wih0p", bufs=1) as wih0p, \
                 tc.tile_pool(name="x0sb", bufs=2) as x0sb, \
                 tc.tile_pool(name="x0psum", bufs=2, space="PSUM") as x0ps, \
                 tc.tile_pool(name="b0p", bufs=1) as b0p:
                wih0_s = wih0p.tile([P, KE, G], F16)
                nc.sync.dma_start(wih0_s, wih0T_v)
                bsum0_s = b0p.tile([1, G], F16)
                nc.sync.dma_start(bsum0_s, bsum0)
                for m in range(TB // P):
                    if "C" not in phases:
                        break
                    for half in range(2):
                        ps = x0ps.tile([P, 4, 512], F32, tag="x0ps")
                        for k in range(KE):
                            for n in range(4):
                                col = half * 2048 + n * 512
                                nc.tensor.matmul(
                                    ps[:, n, :],
                                    embT_s[:, k, m * P : (m + 1) * P],
                                    wih0_s[:, k, col : col + 512],
                                    start=(k == 0),
                                    stop=False,
                                )
                        for n in range(4):
                            col = half * 2048 + n * 512
                            nc.tensor.matmul(
                                ps[:, n, :],
                                ones128,
                                bsum0_s[:, col : col + 512],
                                start=False,
                                stop=True,
                            )
                        xs = x0sb.tile([P, 2048], F16, tag="xs")
                        nc.vector.tensor_copy(xs, ps)
                        nc.sync.dma_start(
                            x0buf[m * P : (m + 1) * P,
                                  half * 2048 : (half + 1) * 2048],
                            xs,
                        )
            abc_pool.release()

            # ---------------- Phase D: recurrence --------------------------
            # Software-pipelined emission order per step keeps the PE fed:
            #   [L0 whh0+X0 MMs] [h1(t-1) transposes] [L1 whh1 half]
            #   [h0(t) transposes] [L1 wih1 half + bias] ...
            # so layer-1 matmuls that depend only on old state cover the
            # latency of layer-0's ACT/DVE elementwise chain.
            with tc.tile_pool(name="wih1p", bufs=2) as wih1p, \
                 tc.tile_pool(name="x0tp", bufs=1) as x0tp, \
                 tc.tile_pool(name="gact", bufs=1) as gact, \
                 tc.tile_pool(name="hsb", bufs=2) as hsbp, \
                 tc.tile_pool(name="gpsum", bufs=3, space="PSUM") as gps, \
                 tc.tile_pool(name="trpsum", bufs=2, space="PSUM") as trps:

                def drain_gate(ps, gp, lay, c_s, acc):
                    """ACT drain of one gate piece; returns via acc dict."""
                    if gp == 3:
                        tg = gact.tile([BL, 1024], F32, tag="tg")
                        nc.scalar.activation(tg, ps, TANH)
                        acc["tanh_g"] = tg
                    elif gp == 1:
                        sig_f = gact.tile([BL, 1024], F32, tag="tg")
                        nc.scalar.activation(sig_f, ps, SIG)
                        nc.vector.tensor_mul(c_s, sig_f, c_s)
                    else:
                        sg = gact.tile([BL, 1024], F32, tag=f"sig{gp}")
                        nc.scalar.activation(sg, ps, SIG)
                        acc["sig_i" if gp == 0 else "sig_o"] = sg

                def elementwise_tail(acc, c_s):
                    """c += sig_i*tanh_g; h = sig_o*tanh(c) -> fp16 tile."""
                    tanh_g, sig_i, sig_o = acc["tanh_g"], acc["sig_i"], acc["sig_o"]
                    nc.vector.tensor_mul(tanh_g, sig_i, tanh_g)
                    nc.vector.tensor_add(c_s, c_s, tanh_g)
                    tc_t = gact.tile([BL, H], F32, tag="tg")
                    nc.scalar.activation(tc_t, c_s, TANH)
                    h_sb = hsbp.tile([BL, H], F16, tag="hsb")
                    nc.vector.tensor_mul(h_sb, sig_o, tc_t)
                    return h_sb

                def transpose_h(h_sb):
                    pt = trps.tile([P, KH, BL], F16, tag="pt")
                    for j in range(KH):
                        nc.tensor.transpose(
                            pt[:, j, :], h_sb[:, j * P : (j + 1) * P], id16
                        )
                    return pt

                h0T_prev = h0T
                h1_sb_prev = None
                for t in range(T):
                    if "D" not in phases:
                        break
                    # ---- L0: gates0 = whh0 @ h0 + X0[t] --------------------
                    acc0 = {}
                    x0t = None
                    for gp in range(4):
                        if gp % 2 == 0:
                            x0t = x0tp.tile([BL, 2048], F16, tag="x0t")
                            nc.sync.dma_start(
                                x0t,
                                x0buf[t * BL : (t + 1) * BL,
                                      (gp // 2) * 2048 : (gp // 2 + 1) * 2048],
                            )
                        ps = gps.tile([BL, 1024], F32, tag="gp")
                        for k in range(KH):
                            for nn in range(2):
                                col = gp * 1024 + nn * 512
                                nc.tensor.matmul(
                                    ps[:, nn * 512 : (nn + 1) * 512],
                                    h0T_prev[:, k, :],
                                    whh0_s[:, k, col : col + 512],
                                    start=(k == 0),
                                    stop=False,
                                )
                        for nn in range(2):
                            hcol = (gp % 2) * 1024 + nn * 512
                            nc.tensor.matmul(
                                ps[:, nn * 512 : (nn + 1) * 512],
                                id16,
                                x0t[:, hcol : hcol + 512],
                                start=False,
                                stop=True,
                            )
                        drain_gate(ps, gp, 0, c0_s, acc0)

                    # ---- h1(t-1) transposes -> ysT[t-1] --------------------
                    if t > 0:
                        pt = transpose_h(h1_sb_prev)
                        nc.vector.tensor_copy(ysT[:, :, t - 1, :], pt)

                    # ---- wih1 quarter prefetches ---------------------------
                    wqs = {}
                    for gp in range(4):
                        wq = wih1p.tile([P, KH, 1024], F16, tag="wq")
                        nc.sync.dma_start(wq, wih1T_v[:, gp, :, :])
                        wqs[gp] = wq

                    # ---- L1 halves: whh1 first (old state), wih1 second ----
                    l1_ps = {}
                    acc1 = {}
                    h0T_new = None
                    for half in range(2):
                        for gp in (2 * half, 2 * half + 1):
                            ps = gps.tile([BL, 1024], F32, tag="gp")
                            l1_ps[gp] = ps
                            for k in range(KH):
                                stat = (
                                    h1T0_s[:, k, :] if t == 0
                                    else ysT[:, k, t - 1, :]
                                )
                                for nn in range(2):
                                    col = gp * 1024 + nn * 512
                                    nc.tensor.matmul(
                                        ps[:, nn * 512 : (nn + 1) * 512],
                                        stat,
                                        whh1_s[:, k, col : col + 512],
                                        start=(k == 0),
                                        stop=False,
                                    )
                        if half == 0:
                            # layer-0 elementwise tail + h0 transposes land
                            # here, covered by the whh1 matmuls above
                            h0_sb = elementwise_tail(acc0, c0_s)
                            pt0 = transpose_h(h0_sb)
                            h0T_new = h0tp.tile([P, KH, BL], F16, tag="h0T")
                            nc.vector.tensor_copy(h0T_new, pt0)
                        for gp in (2 * half, 2 * half + 1):
                            ps = l1_ps[gp]
                            wq = wqs[gp]
                            for k in range(KH):
                                for nn in range(2):
                                    nc.tensor.matmul(
                                        ps[:, nn * 512 : (nn + 1) * 512],
                                        h0T_new[:, k, :],
                                        wq[:, k, nn * 512 : (nn + 1) * 512],
                                        start=False,
                                        stop=False,
                                    )
                            for nn in range(2):
                                col = gp * 1024 + nn * 512
                                nc.tensor.matmul(
                                    ps[:, nn * 512 : (nn + 1) * 512],
                                    ones16,
                                    bsum1_s[:, col : col + 512],
                                    start=False,
                                    stop=True,
                                )
                            drain_gate(ps, gp, 1, c1_s, acc1)

                    h1_sb_prev = elementwise_tail(acc1, c1_s)
                    h0T_prev = h0T_new

                if "D" in phases:
                    pt = transpose_h(h1_sb_prev)
                    nc.vector.tensor_copy(ysT[:, :, T - 1, :], pt)

            # ---------------- Phase E: FC over vocab -----------------------
            with tc.tile_pool(name="fcw", bufs=3) as fcwp, \
                 tc.tile_pool(name="fcb", bufs=2) as fcbp, \
                 tc.tile_pool(name="fcout", bufs=3) as fcoutp, \
                 tc.tile_pool(name="fcpsum", bufs=4, space="PSUM") as fcps:
                nvt = (V + 511) // 512 if "E" in phases else 0
                for vt in range(nvt):
                    w = min(512, V - vt * 512)
                    fcw_t = fcwp.tile([P, KH, 512], F16, tag="fcw")
                    nc.sync.dma_start(
                        fcw_t[:, :, :w], fcwT_v[:, :, vt * 512 : vt * 512 + w]
                    )
                    fcb_t = fcbp.tile([P, 512], F32, tag="fcb")
                    nc.sync.dma_start(
                        fcb_t[:, :w], fcb_rep[:, vt * 512 : vt * 512 + w]
                    )
                    tpm = P // BL  # timesteps per 128-row output chunk
                    for m in range(TB // P):
                        ps = fcps.tile([P, 512], F32, tag="fcps")
                        for k in range(KH):
                            nc.tensor.matmul(
                                ps[:, :w],
                                ysT[:, k, m * tpm : (m + 1) * tpm, :],
                                fcw_t[:, k, :w],
                                start=(k == 0),
                                stop=(k == KH - 1),
                            )
                        ot = fcoutp.tile([P, 512], F32, tag="fcout")
                        nc.vector.tensor_add(ot[:, :w], ps[:, :w], fcb_t[:, :w])
                        nc.sync.dma_start(
                            out[m * P : (m + 1) * P, vt * 512 : vt * 512 + w],
                            ot[:, :w],
                        )

    nc.finalize()
    return nc


def _get_compiled():
    if "nc" not in _cache:
        _cache["nc"] = _build_nc()
    return _cache["nc"]


def _prep_inputs(features, captions, embed_table, init_h_w, init_h_b,
                 init_c_w, init_c_b, w_ih0, w_hh0, b_ih0, b_hh0,
                 w_ih1, w_hh1, b_ih1, b_hh1, fc_w, fc_b):
    f16 = lambda x: np.ascontiguousarray(np.asarray(x), dtype=np.float32).astype(np.float16)
    f32 = lambda x: np.ascontiguousarray(np.asarray(x), dtype=np.float32)

    gperm = [0, 1, 3, 2]  # i, f, g, o -> i, f, o, g

    def gate_permute_T(wmat):
        wmat = np.asarray(wmat, dtype=np.float32)
        k = wmat.shape[1]
        return np.ascontiguousarray(
            wmat.reshape(4, H, k)[gperm].reshape(G, k).T
        ).astype(np.float16)

    def gate_permute_b(b1, b2):
        s = (np.asarray(b1, np.float32) + np.asarray(b2, np.float32))
        return s.reshape(4, H)[gperm].reshape(1, G).astype(np.float16)

    def init_permute_T(wmat):
        # rows r = h*L + l  ->  layer-major rows l*H + h, then transpose
        wmat = np.asarray(wmat, dtype=np.float32)
        return np.ascontiguousarray(
            wmat.reshape(H, L, F).transpose(1, 0, 2).reshape(HL, F).T
        ).astype(np.float16)

    def init_permute_b(bvec):
        bvec = np.asarray(bvec, dtype=np.float32)
        return bvec.reshape(H, L).T.reshape(1, HL).astype(np.float16)

    shared = {
        "table": f16(embed_table),
        "init_hw": init_permute_T(init_h_w),
        "init_cw": init_permute_T(init_c_w),
        "init_hb": init_permute_b(init_h_b),
        "init_cb": init_permute_b(init_c_b),
        "wih0T": gate_permute_T(w_ih0),
        "whh0T": gate_permute_T(w_hh0),
        "wih1T": np.ascontiguousarray(
            gate_permute_T(w_ih1).reshape(H, 4, 1024)
            .transpose(1, 0, 2).reshape(G, 1024)
        ),
        "whh1T": gate_permute_T(w_hh1),
        "bsum0": gate_permute_b(b_ih0, b_hh0),
        "bsum1": gate_permute_b(b_ih1, b_hh1),
        "fcwT": np.ascontiguousarray(
            np.asarray(fc_w, dtype=np.float32).T
        ).astype(np.float16),
        "fcb_rep": np.ascontiguousarray(
            np.broadcast_to(np.asarray(fc_b, np.float32), (P, V))
        ),
    }

    features = np.asarray(features, dtype=np.float32)
    captions = np.asarray(captions).astype(np.int32)

    in_maps = []
    for c in range(NCORES):
        bsl = slice(c * BL, (c + 1) * BL)
        m = dict(shared)
        m["featT"] = np.ascontiguousarray(features[bsl].T).astype(np.float16)
        # row r = t*BL + b  ->  captions[b_global, t]
        m["emb_idx"] = np.ascontiguousarray(
            captions[bsl].T.reshape(TB, 1)
        )
        in_maps.append(m)
    return in_maps


last_results = None


def kernel(**inputs) -> np.ndarray:
    global last_results
    nc = _get_compiled()
    in_maps = _prep_inputs(**inputs)
    res = run_bass_kernel_spmd(nc, in_maps, core_ids=list(range(NCORES)))
    last_results = res
    parts = [res.results[c]["out"].reshape(T, BL, V) for c in range(NCORES)]
    return np.concatenate(parts, axis=1)



# revision 9
# speedup vs baseline: 1.6610x; 1.6610x over previous
"""Trainium2 Bass kernel for a 2-layer LSTM decoder (B=128, T=32, F=2048,
E=512, H=1024, V=10000), gate-TENSOR-parallel across 8 NeuronCores.

Sharding (vs. the data-parallel hint): each core owns a 1/8 slice of the
hidden dim (HSL=128) => a 512-col gate slice [i|f|o|g] of every LSTM
weight, the matching slice of the cell state, and a 1250-col vocab slice
of the FC layer. The full batch B=128 is the matmul stationary operand on
every core, so the PE array runs at full width (vs 16/128 for DP) and the
serial weight-stream cost of the recurrence is sharded 8x.

Per step the cores exchange hidden-state slices with ONE 8-way AllGather
of [h0T(t) ; h1T(t-1)] (layer-1 runs one step skewed so both slices ride
the same collective). FC(t) consumes the gathered h1T(t) two iterations
later, so its matmuls fill the PE while the collective is in flight.

All matmul operands fp16 (1 PE row/cycle), fp32 PSUM + cell state.
Biases fold in via K=1 ones-row matmuls; X0 = emb @ wih0_slice + b0 is
precomputed into SBUF for all 32 steps.
"""

import contextlib

import numpy as np

import concourse.bass as bass
import concourse.mybir as mybir
from concourse import bacc
from concourse.bass_utils import run_bass_kernel_spmd
from concourse.masks import make_identity
from concourse.tile import TileContext

P = 128
NCORES = 8
B, T, F, E, H, L, V = 128, 32, 2048, 512, 1024, 2, 10000
G = 4 * H
TB = T * B                 # 4096 output rows per core, t-major
HSL = H // NCORES          # 128 hidden cols per core
GSL = 4 * HSL              # 512 gate cols per core (i,f,o,g x 128)
VSL = V // NCORES          # 1250 vocab cols per core
KF, KE, KH = F // P, E // P, H // P      # 16, 4, 8
F16 = mybir.dt.float16
F32 = mybir.dt.float32
RG = [list(range(NCORES))]

_cache = {}


def _build_nc():
    nc = bacc.Bacc("TRN2", target_bir_lowering=False, debug=False,
                   enable_asserts=False, num_devices=NCORES)

    def din(name, shape, dt=F16):
        return nc.dram_tensor(name, shape, dt, kind="ExternalInput").ap()

    table = din("table", [V, E])
    emb_idx = din("emb_idx", [TB, 1], mybir.dt.int32)
    featT = din("featT", [F, B])
    initw_h = din("initw_h", [F, 2 * HSL])
    initw_c = din("initw_c", [F, 2 * HSL])
    initb_h = din("initb_h", [1, 2 * HSL])
    initb_c = din("initb_c", [1, 2 * HSL])
    wih0T = din("wih0T", [E, GSL])
    whh0T = din("whh0T", [H, GSL])
    wih1T = din("wih1T", [H, GSL])
    whh1T = din("whh1T", [H, GSL])
    bsum0 = din("bsum0", [1, GSL])
    bsum1 = din("bsum1", [1, GSL])
    fcwT = din("fcwT", [H, VSL])
    fcb_rep = din("fcb_rep", [P, VSL], F32)
    out = nc.dram_tensor("out", [TB, VSL], F32, kind="ExternalOutput").ap()

    featT_v = featT.rearrange("(k p) b -> p k b", p=P)
    initw_h_v = initw_h.rearrange("(k p) n -> p k n", p=P)
    initw_c_v = initw_c.rearrange("(k p) n -> p k n", p=P)
    wih0T_v = wih0T.rearrange("(k p) g -> p k g", p=P)
    whh0T_v = whh0T.rearrange("(k p) g -> p k g", p=P)
    wih1T_v = wih1T.rearrange("(k p) g -> p k g", p=P)
    whh1T_v = whh1T.rearrange("(k p) g -> p k g", p=P)
    fcwT_v = fcwT.rearrange("(k p) v -> p k v", p=P)
    idx_v = emb_idx.rearrange("(g p) one -> p g one", p=P)

    SIG = mybir.ActivationFunctionType.Sigmoid
    TANH = mybir.ActivationFunctionType.Tanh

    with TileContext(nc) as tc, \
         tc.tile_pool(name="const", bufs=1) as constp, \
         tc.tile_pool(name="resident", bufs=1) as resp, \
         tc.tile_pool(name="state", bufs=1) as statep, \
         tc.tile_pool(name="ccd", bufs=3, space="DRAM") as ccp, \
         tc.tile_pool(name="stg", bufs=3) as stgp, \
         tc.tile_pool(name="hg", bufs=3) as hgp:

        id128 = constp.tile([P, P], F16)
        make_identity(nc, id128)
        ones128 = constp.tile([1, P], F16)
        nc.gpsimd.memset(ones128, 1.0)
        bsum0_s = constp.tile([1, GSL], F16)
        nc.sync.dma_start(bsum0_s, bsum0)
        bsum1_s = constp.tile([1, GSL], F16)
        nc.sync.dma_start(bsum1_s, bsum1)

        # Streaming weights, SBUF-resident for the whole kernel.
        # Emission order = DMA order = first-use order.
        wih0_s = resp.tile([P, KE, GSL], F16)
        nc.sync.dma_start(wih0_s, wih0T_v)
        whh0_s = resp.tile([P, KH, GSL], F16)
        nc.sync.dma_start(whh0_s, whh0T_v)
        wih1_s = resp.tile([P, KH, GSL], F16)
        nc.sync.dma_start(wih1_s, wih1T_v)
        whh1_s = resp.tile([P, KH, GSL], F16)
        nc.sync.dma_start(whh1_s, whh1T_v)
        fcw_s = resp.tile([P, KH, VSL], F16)
        nc.sync.dma_start(fcw_s, fcwT_v)
        fcb_s = resp.tile([P, VSL], F32)
        nc.sync.dma_start(fcb_s, fcb_rep)
        X0_s = resp.tile([P, T, GSL], F16)     # [b, t, gate-slice]

        c0_s = statep.tile([P, HSL], F32)
        c1_s = statep.tile([P, HSL], F32)
        h1T_init = statep.tile([P, P], F16)    # layer-1 h(-1).T slice

        def do_cc(stg_tile):
            """stg_tile: SBUF [j, 2, b] -> flat AllGather -> [c][j][l][b].

            Flat 1-D collective APs give plain rank-concat semantics
            (multi-dim APs interleave ranks per partition row instead)."""
            cc_in = ccp.tile([2 * P * P], F16, tag="ccin")
            nc.sync.dma_start(
                cc_in.rearrange("(j l b) -> j l b", j=P, l=2), stg_tile)
            cc_out = ccp.tile([NCORES * 2 * P * P], F16, tag="ccout",
                              addr_space="Shared")
            nc.gpsimd.collective_compute(
                "AllGather", mybir.AluOpType.bypass,
                replica_groups=RG, ins=[cc_in[:]], outs=[cc_out[:]],
            )
            return cc_out

        def load_hg(cc_out):
            # Split l=0/l=1 halves so whh0/wih1 (which need only h0_full)
            # start after the first 128KB lands.
            hg = hgp.tile([P, KH, 2, P], F16, tag="hg")
            v = cc_out.rearrange("(c j l b) -> j c l b", c=NCORES, j=P, l=2)
            nc.sync.dma_start(hg[:, :, 0, :], v[:, :, 0, :])
            nc.sync.dma_start(hg[:, :, 1, :], v[:, :, 1, :])
            return hg

        # ---------------- Phase A: embedding gather + transpose ---------
        embp_pool = tc.alloc_tile_pool(name="embT", bufs=1)
        embT_s = embp_pool.tile([P, KE, TB], F16)
        with tc.tile_pool(name="embg", bufs=3) as embg, \
             tc.tile_pool(name="embps", bufs=2, space="PSUM") as embps:
            for g in range(TB // P):
                idx_t = embg.tile([P, 1, 1], mybir.dt.int32, tag="idx")
                nc.sync.dma_start(idx_t, idx_v[:, g:g + 1, :])
                rows = embg.tile([P, E], F16, tag="rows")
                nc.gpsimd.indirect_dma_start(
                    out=rows[:], out_offset=None, in_=table[:],
                    in_offset=bass.IndirectOffsetOnAxis(ap=idx_t[:, 0, :],
                                                        axis=0),
                )
                pt = embps.tile([P, KE, P], F16, tag="pt")
                for ke in range(KE):
                    nc.tensor.transpose(pt[:, ke, :],
                                        rows[:, ke * P:(ke + 1) * P], id128)
                nc.vector.tensor_copy(embT_s[:, :, g * P:(g + 1) * P], pt)

        # ---------------- Phase B: h0/c0 init slices + init CC ----------
        stg0 = stgp.tile([P, 2, P], F16, tag="stg")
        with tc.tile_pool(name="initw", bufs=3) as initwp, \
             tc.tile_pool(name="initsb", bufs=1) as initsb, \
             tc.tile_pool(name="initps", bufs=2, space="PSUM") as initps, \
             tc.tile_pool(name="btr", bufs=2, space="PSUM") as btrps:
            featT_s = initsb.tile([P, KF, B], F16, tag="ft")
            nc.sync.dma_start(featT_s, featT_v)
            ihb_s = initsb.tile([1, 2 * HSL], F16, tag="ihb")
            nc.sync.dma_start(ihb_s, initb_h)
            icb_s = initsb.tile([1, 2 * HSL], F16, tag="icb")
            nc.sync.dma_start(icb_s, initb_c)
            for which, (wv, bias_s) in enumerate(
                    ((initw_h_v, ihb_s), (initw_c_v, icb_s))):
                ps = initps.tile([P, 2 * HSL], F32, tag="ips")
                for k in range(KF):
                    wc = initwp.tile([P, 1, 2 * HSL], F16, tag="iwc")
                    nc.sync.dma_start(wc, wv[:, k:k + 1, :])
                    nc.tensor.matmul(ps, featT_s[:, k, :], wc[:, 0, :],
                                     start=(k == 0), stop=False)
                nc.tensor.matmul(ps, ones128, bias_s, start=False, stop=True)
                if which == 0:
                    hh = initsb.tile([P, 2 * HSL], F16, tag="hh")
                    nc.vector.tensor_copy(hh, ps)
                    pt = btrps.tile([P, 2, P], F16, tag="pt")
                    for lay in range(L):
                        nc.tensor.transpose(pt[:, lay, :],
                                            hh[:, lay * P:(lay + 1) * P],
                                            id128)
                    nc.vector.tensor_copy(stg0, pt)
                    nc.vector.tensor_copy(h1T_init, pt[:, 1, :])
                else:
                    nc.vector.tensor_copy(c0_s, ps[:, 0:HSL])
                    nc.vector.tensor_copy(c1_s, ps[:, HSL:2 * HSL])
        cc_prev = do_cc(stg0)

        # ---------------- Phase C: X0 = emb @ wih0_slice + b0 -----------
        with tc.tile_pool(name="x0ps", bufs=3, space="PSUM") as x0ps:
            for m in range(TB // P):     # block m == timestep m (B == P)
                psx = x0ps.tile([P, GSL], F32, tag="x")
                for ke in range(KE):
                    nc.tensor.matmul(psx, embT_s[:, ke, m * P:(m + 1) * P],
                                     wih0_s[:, ke, :],
                                     start=(ke == 0), stop=False)
                nc.tensor.matmul(psx, ones128, bsum0_s, start=False,
                                 stop=True)
                nc.vector.tensor_copy(X0_s[:, m, :], psx)
        embp_pool.release()

        # ---------------- Phase D: recurrence + interleaved FC ----------
        with tc.tile_pool(name="gps", bufs=2, space="PSUM") as gps, \
             tc.tile_pool(name="fcps", bufs=2, space="PSUM") as fcps, \
             tc.tile_pool(name="trps", bufs=2, space="PSUM") as trps, \
             tc.tile_pool(name="act", bufs=2) as actp, \
             tc.tile_pool(name="hsb", bufs=2) as hsbp, \
             tc.tile_pool(name="fco", bufs=3) as fcop:

            def lstm_tail(ps, c_s, tagpfx):
                """gates psum [b, i|f|o|g] -> h slice [b, j] f16."""
                sig = actp.tile([P, 3 * HSL], F32, tag=tagpfx + "sig")
                nc.scalar.activation(sig, ps[:, 0:3 * HSL], SIG)
                tg = actp.tile([P, HSL], F32, tag=tagpfx + "tg")
                nc.scalar.activation(tg, ps[:, 3 * HSL:4 * HSL], TANH)
                nc.vector.tensor_mul(c_s, sig[:, HSL:2 * HSL], c_s)
                nc.vector.tensor_mul(tg, sig[:, 0:HSL], tg)
                nc.vector.tensor_add(c_s, c_s, tg)
                tct = actp.tile([P, HSL], F32, tag=tagpfx + "tc")
                nc.scalar.activation(tct, c_s, TANH)
                h_sb = hsbp.tile([P, HSL], F16, tag=tagpfx + "h")
                nc.vector.tensor_mul(h_sb, sig[:, 2 * HSL:3 * HSL], tct)
                return h_sb

            def fc_chunk(t, lo, hi, hg):
                w = hi - lo
                psf = fcps.tile([P, 512], F32, tag="fc")
                for k in range(KH):
                    nc.tensor.matmul(psf[:, :w], hg[:, k, 1, :],
                                     fcw_s[:, k, lo:hi],
                                     start=(k == 0), stop=(k == KH - 1))
                ot = fcop.tile([P, 512], F32, tag="fco")
                nc.vector.tensor_add(ot[:, :w], psf[:, :w], fcb_s[:, lo:hi])
                nc.sync.dma_start(out[t * P:(t + 1) * P, lo:hi], ot[:, :w])

            for i in range(T + 2):       # L0 step i, L1 step i-1, FC i-2
                have_l0 = i < T
                have_l1 = 1 <= i <= T
                have_fc = i >= 2
                hg = load_hg(cc_prev) if i <= T + 1 else hg

                if have_l0:
                    ps0 = gps.tile([P, GSL], F32, tag="g0")
                    for k in range(KH):
                        nc.tensor.matmul(ps0, hg[:, k, 0, :], whh0_s[:, k, :],
                                         start=(k == 0), stop=False)
                    nc.tensor.matmul(ps0, id128, X0_s[:, i, :],
                                     start=False, stop=True)
                if have_l1:
                    ps1 = gps.tile([P, GSL], F32, tag="g1")
                    for k in range(KH):
                        nc.tensor.matmul(ps1, hg[:, k, 0, :], wih1_s[:, k, :],
                                         start=(k == 0), stop=False)
                    for k in range(KH):
                        nc.tensor.matmul(ps1, hg[:, k, 1, :], whh1_s[:, k, :],
                                         start=False, stop=False)
                    nc.tensor.matmul(ps1, ones128, bsum1_s,
                                     start=False, stop=True)

                # FC part A fills the PE while ACT/DVE drain the gates.
                if have_fc:
                    fc_chunk(i - 2, 0, 512, hg)

                if i <= T:
                    stg = stgp.tile([P, 2, P], F16, tag="stg")
                    pt = trps.tile([P, 2, P], F16, tag="pt")
                    if have_l0:
                        h0_sb = lstm_tail(ps0, c0_s, "l0")
                        nc.tensor.transpose(pt[:, 0, :], h0_sb, id128)
                        nc.vector.tensor_copy(stg[:, 0, :], pt[:, 0, :])
                    if have_l1:
                        h1_sb = lstm_tail(ps1, c1_s, "l1")
                        nc.tensor.transpose(pt[:, 1, :], h1_sb, id128)
                        nc.vector.tensor_copy(stg[:, 1, :], pt[:, 1, :])
                        if not have_l0:  # i == T: l0 half unused, fill
                            nc.vector.tensor_copy(stg[:, 0, :], pt[:, 1, :])
                    else:                # i == 0: ship init h1 slice
                        nc.vector.tensor_copy(stg[:, 1, :], h1T_init)
                    cc_prev = do_cc(stg)

                # FC parts B/C fill the PE while the AllGather flies.
                if have_fc:
                    fc_chunk(i - 2, 512, 1024, hg)
                    fc_chunk(i - 2, 1024, VSL, hg)

    nc.finalize()
    return nc


def _get_compiled():
    if "nc" not in _cache:
        _cache["nc"] = _build_nc()
    return _cache["nc"]


def _prep_inputs(features, captions, embed_table, init_h_w, init_h_b,
                 init_c_w, init_c_b, w_ih0, w_hh0, b_ih0, b_hh0,
                 w_ih1, w_hh1, b_ih1, b_hh1, fc_w, fc_b):
    f32 = lambda x: np.asarray(x, dtype=np.float32)
    f16 = lambda x: np.ascontiguousarray(np.asarray(x, dtype=np.float32)
                                         ).astype(np.float16)

    w_ih0, w_hh0, w_ih1, w_hh1 = map(f32, (w_ih0, w_hh0, w_ih1, w_hh1))
    init_h_w, init_c_w = f32(init_h_w), f32(init_c_w)
    b0 = f32(b_ih0) + f32(b_hh0)
    b1 = f32(b_ih1) + f32(b_hh1)
    init_h_b, init_c_b = f32(init_h_b), f32(init_c_b)
    fc_w, fc_b = f32(fc_w), f32(fc_b)
    features = f32(features)
    captions = np.asarray(captions).astype(np.int32)

    shared = {
        "table": f16(embed_table),
        "featT": np.ascontiguousarray(features.T).astype(np.float16),
        "emb_idx": np.ascontiguousarray(captions.T.reshape(TB, 1)),
    }

    in_maps = []
    for c in range(NCORES):
        hc = np.arange(c * HSL, (c + 1) * HSL)
        # torch gate order i,f,g,o in rows; our slice order i,f,o,g
        gsel = np.r_[0 * H + hc, 1 * H + hc, 3 * H + hc, 2 * H + hc]
        isel = np.r_[hc * L + 0, hc * L + 1]   # [layer0 block, layer1 block]
        vsl = slice(c * VSL, (c + 1) * VSL)
        m = dict(shared)
        m["whh0T"] = np.ascontiguousarray(w_hh0[gsel].T).astype(np.float16)
        m["wih1T"] = np.ascontiguousarray(w_ih1[gsel].T).astype(np.float16)
        m["whh1T"] = np.ascontiguousarray(w_hh1[gsel].T).astype(np.float16)
        m["wih0T"] = np.ascontiguousarray(w_ih0[gsel].T).astype(np.float16)
        m["bsum0"] = b0[gsel][None, :].astype(np.float16)
        m["bsum1"] = b1[gsel][None, :].astype(np.float16)
        m["initw_h"] = np.ascontiguousarray(init_h_w[isel].T).astype(np.float16)
        m["initw_c"] = np.ascontiguousarray(init_c_w[isel].T).astype(np.float16)
        m["initb_h"] = init_h_b[isel][None, :].astype(np.float16)
        m["initb_c"] = init_c_b[isel][None, :].astype(np.float16)
        m["fcwT"] = np.ascontiguousarray(fc_w[vsl].T).astype(np.float16)
        m["fcb_rep"] = np.ascontiguousarray(
            np.broadcast_to(fc_b[vsl], (P, VSL))).astype(np.float32)
        in_maps.append(m)
    return in_maps


last_results = None


def kernel(**inputs) -> np.ndarray:
    global last_results
    nc = _get_compiled()
    in_maps = _prep_inputs(**inputs)
    res = run_bass_kernel_spmd(nc, in_maps, core_ids=list(range(NCORES)))
    last_results = res
    parts = [res.results[c]["out"].reshape(T, B, VSL) for c in range(NCORES)]
    return np.concatenate(parts, axis=2)


# revision 10
# speedup vs baseline: 1.8899x; 1.1378x over previous
"""Trainium2 Bass kernel for a 2-layer LSTM decoder (B=128, T=32, F=2048,
E=512, H=1024, V=10000), gate-TENSOR-parallel across 8 NeuronCores.

Sharding (vs. the data-parallel hint): each core owns a 1/8 slice of the
hidden dim (HSL=128) => a 512-col gate slice [i|f|o|g] of every LSTM
weight, the matching slice of the cell state, and a 1250-col vocab slice
of the FC layer. The full batch B=128 is the matmul stationary operand on
every core, so the PE array runs at full width (vs 16/128 for DP) and the
serial weight-stream cost of the recurrence is sharded 8x.

Per step the cores exchange hidden-state slices with ONE 8-way AllGather
of [h0T(t) ; h1T(t-1)] (layer-1 runs one step skewed so both slices ride
the same collective; flat 1-D collective APs give rank-concat order).
The ~13us collective latency per step is hidden behind FC(t-2) matmuls,
the embedding gather/transpose for step t+2, and X0(t+2) = emb @ wih0
precompute, all of which are emitted after the collective doorbell so
the PE never idles (idle PE also downclocks, making restarts slower).

All matmul operands fp16 (1 PE row/cycle), fp32 PSUM + cell state.
Biases fold in via K=1 ones-row matmuls.
"""

import numpy as np

import concourse.bass as bass
import concourse.mybir as mybir
from concourse import bacc
from concourse.bass_utils import run_bass_kernel_spmd
from concourse.masks import make_identity
from concourse.tile import TileContext

P = 128
NCORES = 8
B, T, F, E, H, L, V = 128, 32, 2048, 512, 1024, 2, 10000
G = 4 * H
TB = T * B                 # 4096 output rows per core, t-major
HSL = H // NCORES          # 128 hidden cols per core
GSL = 4 * HSL              # 512 gate cols per core (i,f,o,g x 128)
VSL = V // NCORES          # 1250 vocab cols per core
KF, KE, KH = F // P, E // P, H // P      # 16, 4, 8
F16 = mybir.dt.float16
F32 = mybir.dt.float32
RG = [list(range(NCORES))]

_cache = {}


def _build_nc():
    nc = bacc.Bacc("TRN2", target_bir_lowering=False, debug=False,
                   enable_asserts=False, num_devices=NCORES)

    def din(name, shape, dt=F16):
        return nc.dram_tensor(name, shape, dt, kind="ExternalInput").ap()

    table = din("table", [V, E])
    emb_idx = din("emb_idx", [TB, 1], mybir.dt.int32)
    featT = din("featT", [F, B])
    initw_h = din("initw_h", [F, 2 * HSL])
    initw_c = din("initw_c", [F, 2 * HSL])
    initb_h = din("initb_h", [1, 2 * HSL])
    initb_c = din("initb_c", [1, 2 * HSL])
    wih0T = din("wih0T", [E, GSL])
    whh0T = din("whh0T", [H, GSL])
    wih1T = din("wih1T", [H, GSL])
    whh1T = din("whh1T", [H, GSL])
    bsum0 = din("bsum0", [1, GSL])
    bsum1 = din("bsum1", [1, GSL])
    fcwT = din("fcwT", [H, VSL])
    fcb_rep = din("fcb_rep", [P, VSL], F32)
    out = nc.dram_tensor("out", [TB, VSL], F32, kind="ExternalOutput").ap()

    featT_v = featT.rearrange("(k p) b -> p k b", p=P)
    initw_h_v = initw_h.rearrange("(k p) n -> p k n", p=P)
    initw_c_v = initw_c.rearrange("(k p) n -> p k n", p=P)
    wih0T_v = wih0T.rearrange("(k p) g -> p k g", p=P)
    whh0T_v = whh0T.rearrange("(k p) g -> p k g", p=P)
    wih1T_v = wih1T.rearrange("(k p) g -> p k g", p=P)
    whh1T_v = whh1T.rearrange("(k p) g -> p k g", p=P)
    fcwT_v = fcwT.rearrange("(k p) v -> p k v", p=P)
    idx_v = emb_idx.rearrange("(g p) one -> p g one", p=P)

    SIG = mybir.ActivationFunctionType.Sigmoid
    TANH = mybir.ActivationFunctionType.Tanh

    with TileContext(nc) as tc, \
         tc.tile_pool(name="const", bufs=1) as constp, \
         tc.tile_pool(name="resident", bufs=1) as resp, \
         tc.tile_pool(name="state", bufs=1) as statep, \
         tc.tile_pool(name="ccd", bufs=3, space="DRAM") as ccp, \
         tc.tile_pool(name="stg", bufs=3) as stgp, \
         tc.tile_pool(name="hg", bufs=3) as hgp, \
         tc.tile_pool(name="embg", bufs=4) as embg, \
         tc.tile_pool(name="embt", bufs=3) as embtp, \
         tc.tile_pool(name="act", bufs=2) as actp, \
         tc.tile_pool(name="hsb", bufs=2) as hsbp, \
         tc.tile_pool(name="fco", bufs=3) as fcop, \
         tc.tile_pool(name="gps", bufs=2, space="PSUM") as gps, \
         tc.tile_pool(name="fcps", bufs=2, space="PSUM") as fcps, \
         tc.tile_pool(name="trps", bufs=2, space="PSUM") as trps:

        # ---- constants / small DMAs -----------------------------------
        id128 = constp.tile([P, P], F16)
        make_identity(nc, id128)
        ones128 = constp.tile([1, P], F16)
        nc.gpsimd.memset(ones128, 1.0)
        bsum0_s = constp.tile([1, GSL], F16)
        nc.sync.dma_start(bsum0_s, bsum0)
        bsum1_s = constp.tile([1, GSL], F16)
        nc.sync.dma_start(bsum1_s, bsum1)

        # ---- DMAs needed first: init matmul operands ------------------
        featT_s = resp.tile([P, KF, B], F16)
        nc.sync.dma_start(featT_s, featT_v)
        ihb_s = constp.tile([1, 2 * HSL], F16)
        nc.sync.dma_start(ihb_s, initb_h)
        icb_s = constp.tile([1, 2 * HSL], F16)
        nc.sync.dma_start(icb_s, initb_c)
        initw_s = resp.tile([P, KF, 2, 2 * HSL], F16)   # [.,k,(h|c),cols]
        nc.sync.dma_start(initw_s[:, :, 0, :], initw_h_v)
        nc.sync.dma_start(initw_s[:, :, 1, :], initw_c_v)

        # ---- embedding gathers for blocks 0..2 (gpsimd queue) ---------
        def gather_block(g):
            idx_t = embg.tile([P, 1, 1], mybir.dt.int32, tag="idx")
            nc.sync.dma_start(idx_t, idx_v[:, g:g + 1, :])
            rows = embg.tile([P, E], F16, tag="rows")
            nc.gpsimd.indirect_dma_start(
                out=rows[:], out_offset=None, in_=table[:],
                in_offset=bass.IndirectOffsetOnAxis(ap=idx_t[:, 0, :],
                                                    axis=0),
            )
            return rows

        rows_q = {}
        for g in range(3):
            rows_q[g] = gather_block(g)

        # ---- recurrence weights (needed from iter 0/1) ----------------
        wih0_s = resp.tile([P, KE, GSL], F16)
        nc.sync.dma_start(wih0_s, wih0T_v)
        whh0_s = resp.tile([P, KH, GSL], F16)
        nc.sync.dma_start(whh0_s, whh0T_v)
        wih1_s = resp.tile([P, KH, GSL], F16)
        nc.sync.dma_start(wih1_s, wih1T_v)
        whh1_s = resp.tile([P, KH, GSL], F16)
        nc.sync.dma_start(whh1_s, whh1T_v)

        X0_s = resp.tile([P, T, GSL], F16)     # [b, t, gate-slice]
        c0_s = statep.tile([P, HSL], F32)
        c1_s = statep.tile([P, HSL], F32)
        h1T_init = statep.tile([P, P], F16)    # layer-1 h(-1).T slice

        def do_cc(stg_tile):
            """stg_tile: SBUF [j, 2, b] -> flat AllGather -> [c][j][l][b]."""
            cc_in = ccp.tile([2 * P * P], F16, tag="ccin")
            nc.sync.dma_start(
                cc_in.rearrange("(j l b) -> j l b", j=P, l=2), stg_tile)
            cc_out = ccp.tile([NCORES * 2 * P * P], F16, tag="ccout",
                              addr_space="Shared")
            nc.gpsimd.collective_compute(
                "AllGather", mybir.AluOpType.bypass,
                replica_groups=RG, ins=[cc_in[:]], outs=[cc_out[:]],
            )
            return cc_out

        def load_hg(cc_out):
            # l=0 first so whh0/wih1 (need only h0_full) start earlier.
            hg = hgp.tile([P, KH, 2, P], F16, tag="hg")
            v = cc_out.rearrange("(c j l b) -> j c l b", c=NCORES, j=P, l=2)
            nc.sync.dma_start(hg[:, :, 0, :], v[:, :, 0, :])
            nc.sync.dma_start(hg[:, :, 1, :], v[:, :, 1, :])
            return hg

        def transpose_block(g, rows):
            """rows [r, E] -> embT block [e_part, ke, r] via PE."""
            pt = trps.tile([P, KE, P], F16, tag="pt")
            for ke in range(KE):
                nc.tensor.transpose(pt[:, ke, :], rows[:, ke * P:(ke + 1) * P],
                                    id128)
            ebt = embtp.tile([P, KE, P], F16, tag="ebt")
            nc.vector.tensor_copy(ebt, pt)
            return ebt

        def x0_block(m, ebt):
            """X0[:, m, :] = emb_block_m @ wih0_slice + b0."""
            psx = fcps.tile([P, GSL], F32, tag="fc")
            for ke in range(KE):
                nc.tensor.matmul(psx, ebt[:, ke, :], wih0_s[:, ke, :],
                                 start=(ke == 0), stop=False)
            nc.tensor.matmul(psx, ones128, bsum0_s, start=False, stop=True)
            nc.vector.tensor_copy(X0_s[:, m, :], psx)

        # ---------------- init h0/c0 slices + init CC ------------------
        stg0 = stgp.tile([P, 2, P], F16, tag="stg")
        for which in range(2):
            bias_s = ihb_s if which == 0 else icb_s
            ps = fcps.tile([P, GSL], F32, tag="fc")
            for k in range(KF):
                nc.tensor.matmul(ps[:, 0:2 * HSL], featT_s[:, k, :],
                                 initw_s[:, k, which, :],
                                 start=(k == 0), stop=False)
            nc.tensor.matmul(ps[:, 0:2 * HSL], ones128, bias_s,
                             start=False, stop=True)
            if which == 0:
                hh = hsbp.tile([P, 2 * HSL], F16, tag="hh")
                nc.vector.tensor_copy(hh, ps[:, 0:2 * HSL])
                pt = trps.tile([P, KE, P], F16, tag="pt")
                for lay in range(L):
                    nc.tensor.transpose(pt[:, lay, :],
                                        hh[:, lay * P:(lay + 1) * P], id128)
                nc.vector.tensor_copy(stg0, pt[:, 0:2, :])
                nc.vector.tensor_copy(h1T_init, pt[:, 1, :])
            else:
                nc.vector.tensor_copy(c0_s, ps[:, 0:HSL])
                nc.vector.tensor_copy(c1_s, ps[:, HSL:2 * HSL])
        cc_prev = do_cc(stg0)

        # ---- prologue embT/X0 for steps 0..1 --------------------------
        ebt_q = {}
        for g in range(2):
            ebt_q[g] = transpose_block(g, rows_q.pop(g))
            x0_block(g, ebt_q.pop(g))

        # ---- FC weights (first needed at iter 2) ----------------------
        fcw_s = resp.tile([P, KH, VSL], F16)
        nc.sync.dma_start(fcw_s, fcwT_v)
        fcb_s = resp.tile([P, VSL], F32)
        nc.sync.dma_start(fcb_s, fcb_rep)

        # ---------------- recurrence + interleaved everything ----------
        def lstm_tail(ps, c_s, tagpfx):
            """gates psum [b, i|f|o|g] -> h slice [b, j] f16."""
            sig = actp.tile([P, 3 * HSL], F32, tag=tagpfx + "sig")
            nc.scalar.activation(sig, ps[:, 0:3 * HSL], SIG)
            tg = actp.tile([P, HSL], F32, tag=tagpfx + "tg")
            nc.scalar.activation(tg, ps[:, 3 * HSL:4 * HSL], TANH)
            nc.vector.tensor_mul(c_s, sig[:, HSL:2 * HSL], c_s)
            nc.vector.tensor_mul(tg, sig[:, 0:HSL], tg)
            nc.vector.tensor_add(c_s, c_s, tg)
            tct = actp.tile([P, HSL], F32, tag=tagpfx + "tc")
            nc.scalar.activation(tct, c_s, TANH)
            h_sb = hsbp.tile([P, HSL], F16, tag=tagpfx + "h")
            nc.vector.tensor_mul(h_sb, sig[:, 2 * HSL:3 * HSL], tct)
            return h_sb

        def fc_chunk(t, lo, hi, hg):
            w = hi - lo
            psf = fcps.tile([P, GSL], F32, tag="fc")
            for k in range(KH):
                nc.tensor.matmul(psf[:, :w], hg[:, k, 1, :],
                                 fcw_s[:, k, lo:hi],
                                 start=(k == 0), stop=(k == KH - 1))
            ot = fcop.tile([P, 512], F32, tag="fco")
            nc.vector.tensor_add(ot[:, :w], psf[:, :w], fcb_s[:, lo:hi])
            nc.sync.dma_start(out[t * P:(t + 1) * P, lo:hi], ot[:, :w])

        for i in range(T + 2):           # L0 step i, L1 step i-1, FC i-2
            have_l0 = i < T
            have_l1 = 1 <= i <= T
            have_fc = i >= 2
            if i <= T + 1:
                hg = load_hg(cc_prev)

            if have_l0:
                ps0 = gps.tile([P, GSL], F32, tag="g0")
                for k in range(KH):
                    nc.tensor.matmul(ps0, hg[:, k, 0, :], whh0_s[:, k, :],
                                     start=(k == 0), stop=False)
                nc.tensor.matmul(ps0, id128, X0_s[:, i, :],
                                 start=False, stop=True)
            if have_l1:
                ps1 = gps.tile([P, GSL], F32, tag="g1")
                for k in range(KH):
                    nc.tensor.matmul(ps1, hg[:, k, 0, :], wih1_s[:, k, :],
                                     start=(k == 0), stop=False)
                for k in range(KH):
                    nc.tensor.matmul(ps1, hg[:, k, 1, :], whh1_s[:, k, :],
                                     start=False, stop=False)
                nc.tensor.matmul(ps1, ones128, bsum1_s,
                                 start=False, stop=True)

            # FC part A fills the PE while ACT/DVE drain the gates.
            if have_fc:
                fc_chunk(i - 2, 0, 512, hg)

            if i <= T:
                stg = stgp.tile([P, 2, P], F16, tag="stg")
                pt = trps.tile([P, KE, P], F16, tag="pt")
                if have_l0:
                    h0_sb = lstm_tail(ps0, c0_s, "l0")
                    nc.tensor.transpose(pt[:, 0, :], h0_sb, id128)
                    nc.vector.tensor_copy(stg[:, 0, :], pt[:, 0, :])
                if have_l1:
                    h1_sb = lstm_tail(ps1, c1_s, "l1")
                    nc.tensor.transpose(pt[:, 1, :], h1_sb, id128)
                    nc.vector.tensor_copy(stg[:, 1, :], pt[:, 1, :])
                    if not have_l0:  # i == T: l0 half unused, fill
                        nc.vector.tensor_copy(stg[:, 0, :], pt[:, 1, :])
                else:                # i == 0: ship init h1 slice
                    nc.vector.tensor_copy(stg[:, 1, :], h1T_init)
                cc_prev = do_cc(stg)

            # ---- collective window fill: FC B/C + embT/X0 pipeline ----
            if have_fc:
                fc_chunk(i - 2, 512, 1024, hg)
            if i + 3 < T:
                rows_q[i + 3] = gather_block(i + 3)
            if i + 2 < T:
                ebt = transpose_block(i + 2, rows_q.pop(i + 2))
                x0_block(i + 2, ebt)
            if have_fc:
                fc_chunk(i - 2, 1024, VSL, hg)

    nc.finalize()
    return nc


def _get_compiled():
    if "nc" not in _cache:
        _cache["nc"] = _build_nc()
    return _cache["nc"]


def _prep_inputs(features, captions, embed_table, init_h_w, init_h_b,
                 init_c_w, init_c_b, w_ih0, w_hh0, b_ih0, b_hh0,
                 w_ih1, w_hh1, b_ih1, b_hh1, fc_w, fc_b):
    f32 = lambda x: np.asarray(x, dtype=np.float32)
    f16 = lambda x: np.ascontiguousarray(np.asarray(x, dtype=np.float32)
                                         ).astype(np.float16)

    w_ih0, w_hh0, w_ih1, w_hh1 = map(f32, (w_ih0, w_hh0, w_ih1, w_hh1))
    init_h_w, init_c_w = f32(init_h_w), f32(init_c_w)
    b0 = f32(b_ih0) + f32(b_hh0)
    b1 = f32(b_ih1) + f32(b_hh1)
    init_h_b, init_c_b = f32(init_h_b), f32(init_c_b)
    fc_w, fc_b = f32(fc_w), f32(fc_b)
    features = f32(features)
    captions = np.asarray(captions).astype(np.int32)

    shared = {
        "table": f16(embed_table),
        "featT": np.ascontiguousarray(features.T).astype(np.float16),
        "emb_idx": np.ascontiguousarray(captions.T.reshape(TB, 1)),
    }

    in_maps = []
    for c in range(NCORES):
        hc = np.arange(c * HSL, (c + 1) * HSL)
        # torch gate order i,f,g,o in rows; our slice order i,f,o,g
        gsel = np.r_[0 * H + hc, 1 * H + hc, 3 * H + hc, 2 * H + hc]
        isel = np.r_[hc * L + 0, hc * L + 1]   # [layer0 block, layer1 block]
        vsl = slice(c * VSL, (c + 1) * VSL)
        m = dict(shared)
        m["whh0T"] = np.ascontiguousarray(w_hh0[gsel].T).astype(np.float16)
        m["wih1T"] = np.ascontiguousarray(w_ih1[gsel].T).astype(np.float16)
        m["whh1T"] = np.ascontiguousarray(w_hh1[gsel].T).astype(np.float16)
        m["wih0T"] = np.ascontiguousarray(w_ih0[gsel].T).astype(np.float16)
        m["bsum0"] = b0[gsel][None, :].astype(np.float16)
        m["bsum1"] = b1[gsel][None, :].astype(np.float16)
        m["initw_h"] = np.ascontiguousarray(init_h_w[isel].T).astype(np.float16)
        m["initw_c"] = np.ascontiguousarray(init_c_w[isel].T).astype(np.float16)
        m["initb_h"] = init_h_b[isel][None, :].astype(np.float16)
        m["initb_c"] = init_c_b[isel][None, :].astype(np.float16)
        m["fcwT"] = np.ascontiguousarray(fc_w[vsl].T).astype(np.float16)
        m["fcb_rep"] = np.ascontiguousarray(
            np.broadcast_to(fc_b[vsl], (P, VSL))).astype(np.float32)
        in_maps.append(m)
    return in_maps


last_results = None


def kernel(**inputs) -> np.ndarray:
    global last_results
    nc = _get_compiled()
    in_maps = _prep_inputs(**inputs)
    res = run_bass_kernel_spmd(nc, in_maps, core_ids=list(range(NCORES)))
    last_results = res
    parts = [res.results[c]["out"].reshape(T, B, VSL) for c in range(NCORES)]
    return np.concatenate(parts, axis=2)
